# revision 1
# baseline (speedup 1.0000x reference)
"""DendriticLayer kernel for Trainium2, 8 NeuronCores, tensor-parallel over dendrites.

Math (reference):
  dendrite_out = leaky_relu(x @ (dendrite_W * dendrite_mask).T + dendrite_b)   [256, 16384]
  soma_out     = leaky_relu(dendrite_out @ (soma_W * soma_mask).T + soma_b)    [256, 1024]

Structural facts this kernel exploits (verified at runtime, with a numpy
fallback if they ever fail to hold):
  - setup_inputs() pre-multiplies dendrite_W and soma_W by their masks, so
    W * mask == W bit-exactly; the masks carry no information and are never
    sent to the device.
  - dendrite_b and soma_b are zeros, so the bias adds are no-ops.
  - soma_mask is block-diagonal: neuron n sees exactly dendrites 16n..16n+16.
    Sharding the 16384 dendrite dim into 8 contiguous chunks of 2048 makes
    neurons 128c..128(c+1) local to core c -> no collectives. Further, the
    soma matmul degenerates to a per-dendrite scale (w_flat[d] =
    soma_W[d//16, d]) followed by a segmented sum of 16 -> computed on the
    Vector engine, no PE work and no transposes.

Per-core device schedule (core c):
  stage 1:  Y[b, d] = lrelu(x @ WdT)  via PE matmuls with x as the stationary
            operand (lhsT = x^T k-tile [128i, 128b], moving = Wd^T k-tile
            [128i, 512d]) -> 256 matmuls of N=512 in fp32r (single-pass f32).
            Streaming W is the DMA bottleneck (32 MB/core ~ the roofline).
  stage 2:  Z[b, n] = lrelu(sum_t Y[b, 16n+t] * w_flat[16n+t])  on DVE:
            tensor_mul + segmented tensor_reduce(X).
"""

import sys

import numpy as np

if "/opt/trn_rl_repo" not in sys.path:
    sys.path.insert(0, "/opt/trn_rl_repo")

IN_DIM = 4096
N_SOMA = 16384
N_NEURONS = 1024
BATCH = 256
NCORES = 8
D_SH = N_SOMA // NCORES  # 2048 dendrites per core
N_SH = N_NEURONS // NCORES  # 128 neurons per core
SOMA_FAN = N_SOMA // N_NEURONS  # 16 dendrites per neuron
P = 128
KT = IN_DIM // P  # 32 k-tiles (stage-1 contraction)
NG = 4  # dendrite groups of 512 per core
GW = D_SH // NG  # 512 dendrites per group
KC = 4  # W k-chunks per group (8 k-tiles = 2 MiB per DMA)
KK = KT // KC  # 8
NEG_SLOPE = 0.1

_CACHE: dict = {}


def _build_bass():
    import concourse.mybir as mybir
    import concourse.tile as tile
    from concourse import bacc

    f32 = mybir.dt.float32
    f32r = mybir.dt.float32r  # single-pass f32 matmul (fp32 proper = 2 half-speed passes)
    nc = bacc.Bacc(trn_type="TRN2")

    # DRAM I/O. Layouts (host-side prep in kernel()):
    #   xt[p, k, b]    = x[b, k*128+p]
    #   wd[g, p, k, j] = Wd_shard[g*512+j, k*128+p]
    #   wb[p, d]       = w_flat[d]  (replicated over p; w_flat[d] = Ws[d//16, d])
    #   out[h, p, n]   = Z[h*128+p, n]
    xt = nc.dram_tensor("xt", [P, KT, BATCH], f32r, kind="ExternalInput")
    wd = nc.dram_tensor("wd", [NG, P, KT, GW], f32r, kind="ExternalInput")
    wb = nc.dram_tensor("wb", [P, D_SH], f32, kind="ExternalInput")
    out = nc.dram_tensor("out", [2, P, N_SH], f32, kind="ExternalOutput")

    ADD = mybir.AluOpType.add
    AX = mybir.AxisListType.X

    with tile.TileContext(nc) as tc:
        with (
            tc.tile_pool(name="const", bufs=1) as cpool,
            tc.tile_pool(name="wpool", bufs=8) as wpool,
            tc.tile_pool(name="ypool", bufs=3) as ypool,
            tc.tile_pool(name="ps1", bufs=2, space="PSUM") as ps1,
        ):
            # x^T resident, in 4 chunks so the first matmuls don't wait on
            # the whole 4 MB.
            # Both HWDGE rings (Sync + Activation) are FIFO: issue order =
            # transfer order within a ring. W chunks alternate between the
            # rings so one ring's inter-DMA gap is covered by the other's
            # in-flight transfer; x chunks are interleaved on the scalar
            # ring in consumption order. All compute runs on PE/DVE so no
            # compute op can head-of-line block a DMA ring.
            xc = []
            for c in range(KC):
                t = cpool.tile([P, KK, BATCH], f32r, name=f"xc{c}", tag=f"xc{c}")
                xc.append(t)
            wb_sb = cpool.tile([P, D_SH], f32)
            z_sb = [cpool.tile([P, N_SH], f32, name=f"z{h}", tag=f"z{h}") for h in range(2)]
            nc.sync.dma_start(xc[0][:], xt[:, 0:KK, :])

            MAX = mybir.AluOpType.max
            CPK = 4  # k-tiles per W chunk -> 1 MiB DMAs: the PE burst per
            # chunk (~1.8us) nearly fills the ~2.4us chunk cadence, so PE
            # idle stays well under the ~3.4us HAM re-throttle window and
            # the clock stays at 2.4 GHz.
            NCH = KT // CPK  # 8 chunks per group
            for g in range(NG):
                ps = [ps1.tile([P, GW], f32, name=f"ps{h}_{g}", tag=f"ps{h}") for h in range(2)]
                for kc in range(NCH):
                    i = g * NCH + kc
                    wc = wpool.tile([P, CPK, GW], f32r, name=f"wc{i}", tag="wc")
                    nc.sync.dma_start(wc[:], wd[g, :, kc * CPK : (kc + 1) * CPK, :])
                    # interleave the x chunks / soma weights into the same
                    # FIFO ring right where consumption order needs them
                    if i == 1:
                        nc.sync.dma_start(xc[1][:], xt[:, KK : 2 * KK, :])
                    elif i == 3:
                        nc.sync.dma_start(xc[2][:], xt[:, 2 * KK : 3 * KK, :])
                    elif i == 5:
                        nc.sync.dma_start(xc[3][:], xt[:, 3 * KK : 4 * KK, :])
                    elif i == 7:
                        nc.sync.dma_start(wb_sb[:], wb[:])
                    for kk in range(CPK):
                        k = kc * CPK + kk
                        for h in range(2):
                            nc.tensor.matmul(
                                ps[h][:],
                                xc[k // KK][:, k % KK, h * P : (h + 1) * P],
                                wc[:, kk, :],
                                start=(k == 0),
                                stop=(k == KT - 1),
                            )
                for h in range(2):
                    # leaky_relu(v) == max(v, 0.1*v), computed on DVE so the
                    # ACT engine stays free for DMA issue.
                    t0 = ypool.tile([P, GW], f32, tag="t0")
                    nc.vector.tensor_scalar_mul(t0[:], ps[h][:], NEG_SLOPE)
                    y = ypool.tile([P, GW], f32, tag="y")
                    nc.vector.tensor_tensor(y[:], t0[:], ps[h][:], op=MAX)
                    yw = ypool.tile([P, GW], f32, tag="yw")
                    nc.vector.tensor_mul(
                        yw[:], y[:], wb_sb[:, g * GW : (g + 1) * GW]
                    )
                    nc.vector.tensor_reduce(
                        z_sb[h][:, g * (GW // SOMA_FAN) : (g + 1) * (GW // SOMA_FAN)],
                        yw[:].rearrange("p (n t) -> p n t", t=SOMA_FAN),
                        axis=AX,
                        op=ADD,
                    )

            for h in range(2):
                f0 = cpool.tile([P, N_SH], f32, name=f"f0{h}", tag=f"f0{h}")
                nc.vector.tensor_scalar_mul(f0[:], z_sb[h][:], NEG_SLOPE)
                zf = cpool.tile([P, N_SH], f32, name=f"zf{h}", tag=f"zf{h}")
                nc.vector.tensor_tensor(zf[:], f0[:], z_sb[h][:], op=MAX)
                nc.sync.dma_start(out[h], zf[:])

    nc.finalize()  # Bacc: wait-splitting + register allocation passes
    return nc


def _numpy_fallback(x, dendrite_W, dendrite_b, soma_W, soma_b, dmask, smask):
    def lrelu(v):
        return np.where(v >= 0, v, NEG_SLOPE * v).astype(np.float32)

    y = lrelu(x @ (dendrite_W * dmask).T + dendrite_b)
    return lrelu(y @ (soma_W * smask).T + soma_b)


def _assumptions_hold(dendrite_W, dendrite_b, soma_W, soma_b, dmask, smask):
    # biases must be exactly zero (setup_inputs hardcodes jnp.zeros)
    if dendrite_b.any() or soma_b.any():
        return False
    # spot-check that the weights are pre-masked (setup_inputs multiplies
    # the masks in): W must vanish wherever its mask does.
    dW = dendrite_W[::173, ::97]
    if np.any(dW * (1.0 - dmask[::173, ::97]) != 0.0):
        return False
    sW = soma_W[::89, ::131]
    if np.any(sW * (1.0 - smask[::89, ::131]) != 0.0):
        return False
    # soma_mask must be the block-diagonal kron(eye, ones(16)) pattern
    n_idx = np.arange(0, N_NEURONS, 37)
    d_idx = np.arange(0, N_SOMA, 53)
    expect = (np.floor_divide(d_idx[None, :], SOMA_FAN) == n_idx[:, None]).astype(
        np.float32
    )
    if np.any(smask[np.ix_(n_idx, d_idx)] != expect):
        return False
    return True


def kernel(x, dendrite_W, dendrite_b, soma_W, soma_b, dendrite_mask, soma_mask):
    x = np.asarray(x, dtype=np.float32)
    dendrite_W = np.asarray(dendrite_W, dtype=np.float32)
    dendrite_b = np.asarray(dendrite_b, dtype=np.float32)
    soma_W = np.asarray(soma_W, dtype=np.float32)
    soma_b = np.asarray(soma_b, dtype=np.float32)
    dendrite_mask = np.asarray(dendrite_mask, dtype=np.float32)
    soma_mask = np.asarray(soma_mask, dtype=np.float32)

    if not _assumptions_hold(
        dendrite_W, dendrite_b, soma_W, soma_b, dendrite_mask, soma_mask
    ):
        return _numpy_fallback(
            x, dendrite_W, dendrite_b, soma_W, soma_b, dendrite_mask, soma_mask
        )

    if "nc" not in _CACHE:
        _CACHE["nc"] = _build_bass()
    nc = _CACHE["nc"]

    # x^T, replicated to every core: xt[p, k, b] = x[b, k*128+p]
    xt = np.ascontiguousarray(x.reshape(BATCH, KT, P).transpose(2, 1, 0))

    in_maps = []
    for c in range(NCORES):
        d0 = c * D_SH
        n0 = c * N_SH
        Wd = dendrite_W[d0 : d0 + D_SH]  # [2048, 4096]
        # wd[g, p, k, j] = Wd[g*512+j, k*128+p]
        wd_c = np.ascontiguousarray(Wd.reshape(NG, GW, KT, P).transpose(0, 3, 2, 1))
        # flat soma weights for this core's block: w_flat[d] = Ws[d//16, d]
        Ws = soma_W[n0 : n0 + N_SH, d0 : d0 + D_SH]  # [128, 2048]
        d_idx = np.arange(D_SH)
        w_flat = Ws[d_idx // SOMA_FAN, d_idx]  # [2048]
        wb_c = np.ascontiguousarray(np.broadcast_to(w_flat, (P, D_SH)))
        in_maps.append({"xt": xt, "wd": wd_c, "wb": wb_c})

    from concourse.bass_utils import run_bass_kernel_spmd

    results = run_bass_kernel_spmd(nc, in_maps, core_ids=list(range(NCORES)))
    _CACHE["last_results"] = results

    full = np.empty((BATCH, N_NEURONS), dtype=np.float32)
    for c in range(NCORES):
        full[:, c * N_SH : (c + 1) * N_SH] = results.results[c]["out"].reshape(
            BATCH, N_SH
        )
    return full



# revision 2
# speedup vs baseline: 1.6929x; 1.6929x over previous
"""DendriticLayer kernel for Trainium2, 8 NeuronCores, tensor-parallel over dendrites.

Math (reference):
  dendrite_out = leaky_relu(x @ (dendrite_W * dendrite_mask).T + dendrite_b)   [256, 16384]
  soma_out     = leaky_relu(dendrite_out @ (soma_W * soma_mask).T + soma_b)    [256, 1024]

Structural facts this kernel exploits (verified at runtime, with a numpy
fallback if they ever fail to hold):
  - setup_inputs() pre-multiplies dendrite_W and soma_W by their masks, so
    W * mask == W bit-exactly; the masks carry no information and are never
    sent to the device.
  - dendrite_b and soma_b are zeros, so the bias adds are no-ops.
  - soma_mask is block-diagonal: neuron n sees exactly dendrites 16n..16n+16.
    Sharding the 16384 dendrite dim into 8 contiguous chunks of 2048 makes
    neurons 128c..128(c+1) local to core c -> no collectives. The soma matmul
    degenerates to a per-dendrite scale followed by a segmented sum of 16,
    computed on the Vector engine.

Perf design (v2): the baseline streamed f32 weights and was DMA-bound at
~344 GB/s (37 MiB/core -> 128+ us). Per-core traffic is cut to ~11 MiB:
  - dendrite_W as fp8 e3m4 with a per-dendrite scale s_d = 15.5/max|row|;
    leaky_relu is positively homogeneous, so the dequant folds into the
    soma stage's per-dendrite multiply (wb = w_soma/s_d). Measured exact
    end-to-end rel err vs the f32 reference: 1.19e-2 (< 2e-2 gate).
  - x as bf16 (stationary matmul operand).
  - everything prefetched into SBUF up front (8 MiB W + 2 MiB x + 1 MiB wb
    fit easily); W chunks on the Sync HWDGE ring, x + wb on the Scalar
    ring, both in PE consumption order. The PE then runs one dense warm
    burst: 256 matmuls of N=512 at ~216 ns -> ~55-56 us PE-bound.
"""

import sys

import numpy as np

if "/opt/trn_rl_repo" not in sys.path:
    sys.path.insert(0, "/opt/trn_rl_repo")

IN_DIM = 4096
N_SOMA = 16384
N_NEURONS = 1024
BATCH = 256
NCORES = 8
D_SH = N_SOMA // NCORES  # 2048 dendrites per core
N_SH = N_NEURONS // NCORES  # 128 neurons per core
SOMA_FAN = N_SOMA // N_NEURONS  # 16 dendrites per neuron
P = 128
KT = IN_DIM // P  # 32 k-tiles (stage-1 contraction)
NG = 4  # dendrite groups of 512 per core
GW = D_SH // NG  # 512 dendrites per group
KCH = 4  # W DMA chunks per group
KS = KT // KCH  # 8 k-tiles per W chunk (512 KiB fp8)
NEG_SLOPE = 0.1
F8_MAX = 15.5  # e3m4 max normal

_CACHE: dict = {}


def _build_bass():
    import concourse.mybir as mybir
    import concourse.tile as tile
    from concourse import bacc

    f32 = mybir.dt.float32
    bf16 = mybir.dt.bfloat16
    f8 = mybir.dt.float8e3  # e3m4: 4 mantissa bits
    nc = bacc.Bacc(trn_type="TRN2")

    # DRAM I/O. Layouts (host-side prep in kernel()):
    #   xt[p, k, b]          = x[b, k*128+p]                     (bf16)
    #   wd[g, c, p, s, j]    = q(Wd_shard[g*512+j, (c*8+s)*128+p] * s_row)  (fp8 e3m4)
    #   wb[p, d]             = w_soma_flat[d] / s_row[d]  (replicated over p, f32)
    #   out[h, p, n]         = Z[h*128+p, n]
    xt = nc.dram_tensor("xt", [P, KT, BATCH], bf16, kind="ExternalInput")
    wd = nc.dram_tensor("wd", [NG, KCH, P, KS, GW], f8, kind="ExternalInput")
    wb = nc.dram_tensor("wb", [P, D_SH], f32, kind="ExternalInput")
    out = nc.dram_tensor("out", [2, P, N_SH], f32, kind="ExternalOutput")

    ADD = mybir.AluOpType.add
    MAX = mybir.AluOpType.max
    AX = mybir.AxisListType.X

    with tile.TileContext(nc) as tc:
        with (
            tc.tile_pool(name="const", bufs=1) as cpool,
            tc.tile_pool(name="ypool", bufs=3) as ypool,
            tc.tile_pool(name="ps1", bufs=2, space="PSUM") as ps1,
        ):
            # All inputs prefetched to SBUF. Two independent HWDGE FIFO
            # rings: W chunks (16 x 512 KiB) on Sync in PE consumption
            # order; x chunks + wb (~3 MiB) on Scalar so they never delay
            # W. Total ~11 MiB at ~358 GB/s finishes well inside the PE's
            # ~55 us of matmul work.
            xc = []
            for c in range(KCH):
                t = cpool.tile([P, KS, BATCH], bf16, name=f"xc{c}", tag=f"xc{c}")
                xc.append(t)
            wc = []
            for i in range(NG * KCH):
                t = cpool.tile([P, KS, GW], f8, name=f"wc{i}", tag=f"wc{i}")
                wc.append(t)
            wb_sb = cpool.tile([P, D_SH], f32)
            z_sb = [cpool.tile([P, N_SH], f32, name=f"z{h}", tag=f"z{h}") for h in range(2)]

            for c in range(KCH):
                nc.scalar.dma_start(xc[c][:], xt[:, c * KS : (c + 1) * KS, :])
            nc.scalar.dma_start(wb_sb[:], wb[:])
            for g in range(NG):
                for c in range(KCH):
                    i = g * KCH + c
                    nc.sync.dma_start(wc[i][:], wd[g, c])

            for g in range(NG):
                ps = [ps1.tile([P, GW], f32, name=f"ps{h}_{g}", tag=f"ps{h}") for h in range(2)]
                for k in range(KT):
                    for h in range(2):
                        nc.tensor.matmul(
                            ps[h][:],
                            xc[k // KS][:, k % KS, h * P : (h + 1) * P],
                            wc[g * KCH + k // KS][:, k % KS, :],
                            start=(k == 0),
                            stop=(k == KT - 1),
                        )
                for h in range(2):
                    # leaky_relu(v) == max(v, 0.1*v) on DVE; the fp8 dequant
                    # scale rides along inside wb (positively homogeneous).
                    t0 = ypool.tile([P, GW], f32, tag="t0")
                    nc.vector.tensor_scalar_mul(t0[:], ps[h][:], NEG_SLOPE)
                    y = ypool.tile([P, GW], f32, tag="y")
                    nc.vector.tensor_tensor(y[:], t0[:], ps[h][:], op=MAX)
                    yw = ypool.tile([P, GW], f32, tag="yw")
                    nc.vector.tensor_mul(
                        yw[:], y[:], wb_sb[:, g * GW : (g + 1) * GW]
                    )
                    nc.vector.tensor_reduce(
                        z_sb[h][:, g * (GW // SOMA_FAN) : (g + 1) * (GW // SOMA_FAN)],
                        yw[:].rearrange("p (n t) -> p n t", t=SOMA_FAN),
                        axis=AX,
                        op=ADD,
                    )

            for h in range(2):
                f0 = cpool.tile([P, N_SH], f32, name=f"f0{h}", tag=f"f0{h}")
                nc.vector.tensor_scalar_mul(f0[:], z_sb[h][:], NEG_SLOPE)
                zf = cpool.tile([P, N_SH], f32, name=f"zf{h}", tag=f"zf{h}")
                nc.vector.tensor_tensor(zf[:], f0[:], z_sb[h][:], op=MAX)
                nc.sync.dma_start(out[h], zf[:])

    nc.finalize()  # Bacc: wait-splitting + register allocation passes
    return nc


def _numpy_fallback(x, dendrite_W, dendrite_b, soma_W, soma_b, dmask, smask):
    def lrelu(v):
        return np.where(v >= 0, v, NEG_SLOPE * v).astype(np.float32)

    y = lrelu(x @ (dendrite_W * dmask).T + dendrite_b)
    return lrelu(y @ (soma_W * smask).T + soma_b)


def _assumptions_hold(dendrite_W, dendrite_b, soma_W, soma_b, dmask, smask):
    # biases must be exactly zero (setup_inputs hardcodes jnp.zeros)
    if dendrite_b.any() or soma_b.any():
        return False
    # spot-check that the weights are pre-masked (setup_inputs multiplies
    # the masks in): W must vanish wherever its mask does.
    dW = dendrite_W[::173, ::97]
    if np.any(dW * (1.0 - dmask[::173, ::97]) != 0.0):
        return False
    sW = soma_W[::89, ::131]
    if np.any(sW * (1.0 - smask[::89, ::131]) != 0.0):
        return False
    # soma_mask must be the block-diagonal kron(eye, ones(16)) pattern
    n_idx = np.arange(0, N_NEURONS, 37)
    d_idx = np.arange(0, N_SOMA, 53)
    expect = (np.floor_divide(d_idx[None, :], SOMA_FAN) == n_idx[:, None]).astype(
        np.float32
    )
    if np.any(smask[np.ix_(n_idx, d_idx)] != expect):
        return False
    return True


def kernel(x, dendrite_W, dendrite_b, soma_W, soma_b, dendrite_mask, soma_mask):
    import ml_dtypes

    x = np.asarray(x, dtype=np.float32)
    dendrite_W = np.asarray(dendrite_W, dtype=np.float32)
    dendrite_b = np.asarray(dendrite_b, dtype=np.float32)
    soma_W = np.asarray(soma_W, dtype=np.float32)
    soma_b = np.asarray(soma_b, dtype=np.float32)
    dendrite_mask = np.asarray(dendrite_mask, dtype=np.float32)
    soma_mask = np.asarray(soma_mask, dtype=np.float32)

    if not _assumptions_hold(
        dendrite_W, dendrite_b, soma_W, soma_b, dendrite_mask, soma_mask
    ):
        return _numpy_fallback(
            x, dendrite_W, dendrite_b, soma_W, soma_b, dendrite_mask, soma_mask
        )

    if "nc" not in _CACHE:
        _CACHE["nc"] = _build_bass()
    nc = _CACHE["nc"]

    # x^T, replicated to every core: xt[p, k, b] = x[b, k*128+p]
    xt = np.ascontiguousarray(
        x.reshape(BATCH, KT, P).transpose(2, 1, 0).astype(ml_dtypes.bfloat16)
    )

    in_maps = []
    for c in range(NCORES):
        d0 = c * D_SH
        n0 = c * N_SH
        Wd = dendrite_W[d0 : d0 + D_SH]  # [2048, 4096]
        # per-dendrite fp8 scale: map each row's max to the e3m4 max normal
        rowmax = np.abs(Wd).max(axis=1)
        s_row = np.where(rowmax > 0, F8_MAX / np.maximum(rowmax, 1e-30), 1.0).astype(
            np.float32
        )
        Wq = (Wd * s_row[:, None]).astype(ml_dtypes.float8_e3m4)
        # wd[g, ch, p, s, j] = Wq[g*512+j, ((ch*8+s)*128)+p]
        wd_c = np.ascontiguousarray(
            Wq.reshape(NG, GW, KCH, KS, P).transpose(0, 2, 4, 3, 1)
        )
        # flat soma weights with the fp8 dequant folded in:
        #   wb[d] = soma_W[d//16, d] / s_row[d]
        Ws = soma_W[n0 : n0 + N_SH, d0 : d0 + D_SH]  # [128, 2048]
        d_idx = np.arange(D_SH)
        w_flat = (Ws[d_idx // SOMA_FAN, d_idx] / s_row).astype(np.float32)
        wb_c = np.ascontiguousarray(np.broadcast_to(w_flat, (P, D_SH)))
        in_maps.append({"xt": xt, "wd": wd_c, "wb": wb_c})

    from concourse.bass_utils import run_bass_kernel_spmd

    results = run_bass_kernel_spmd(nc, in_maps, core_ids=list(range(NCORES)))
    _CACHE["last_results"] = results

    full = np.empty((BATCH, N_NEURONS), dtype=np.float32)
    for c in range(NCORES):
        full[:, c * N_SH : (c + 1) * N_SH] = results.results[c]["out"].reshape(
            BATCH, N_SH
        )
    return full
